# revision 27
# baseline (speedup 1.0000x reference)
"""Fused FBP (ramp-filter + backprojection + flip + resize + crop) Trainium2 kernel.

The whole reference pipeline is linear in the input sinogram, so it folds into a
single constant matrix T of shape (A*DET, W*W) = (20736, 9216):

    out[n, p] = sum_k x_flat[n, k] * T[k, p]

T has a 4-fold exact symmetry:
  angle mirror:    T[(215-i, d)]    = mirror_x(T[(i, d)])        (i < 108)
  detector mirror: T[(i, 95-d)]     = rot180(T[(i, d)])          (d < 48)
so only the (i < 108, d < 48) quarter of T is streamed. The output-pixel axis is
sharded across 8 cores as y-mirror-closed row sets L_c = {6c..6c+5} u {90-6c..
95-6c}; each core owns 1152 pixels = 9 blocks of 128.

Mapping: T is the STATIONARY matmul operand (pixel block = PSUM partitions,
M=128, full array) and the four symmetry-variant x packs stream together as one
N=384 moving operand per (k-chunk, block):

    psum[blk][p, 4*96] += T[k, p_blk].T @ [xA | xB | xC | xD][k, :]

41 k-chunks accumulate per block; blocks 0-7 live in PSUM banks 0-7 chunk-outer
(so DMA stays ahead of the PE from the first chunk), block 8 runs as a second
41-chunk pass reusing bank 0 after its copy-out. The host applies the mirror
permutations when combining the four variant outputs:

    out[p] = A[p] + B[mirror_x(p)] + C[rot180(p)] + D[mirror_y(p)]

which is pure numpy indexing and off the measured HW path. PE cost: 369 matmuls
x (384/2.4 + 2.5) ns ~= 60 us vs ~85 us for the x-stationary formulation (the
moving operand there is T itself: 4 uses x 41x1152 columns = 188928 cycles).
"""

import numpy as np
import ml_dtypes

N_ANGLES = 216
DET = 96
WIDTH = 96
UPSAMPLE = 1.8
PAD = 256

SLICES = 96                    # 2*1*48 sinogram slices
K = N_ANGLES * DET             # 20736 full contraction length
P_TOTAL = WIDTH * WIDTH        # 9216 output pixels per slice
NCORES = 8
PSH = P_TOTAL // NCORES        # 1152 output pixels per core
NBLK = PSH // 128              # 9 pixel blocks of 128 per core
A_HALF = N_ANGLES // 2         # 108
D_HALF = DET // 2              # 48
KQ = A_HALF * D_HALF           # 5184 quarter rows
KCQ = (KQ + 127) // 128        # 41 k-chunks (last one zero-padded)
KQP = KCQ * 128                # 5248 padded rows
NV = 4                         # symmetry variants A,B,C,D
NMOV = NV * SLICES             # 384 moving columns per matmul

# The x pack and T slab for each chunk are fused into one DRAM row of
# NMOV+PSH=1536 bf16 cols, so one dma_start supplies a whole chunk and
# arrivals exactly track consumption. Per-chunk DMAs early (fine-grained
# pipeline), 4-chunk groups later (queue overhead amortized).
CHUNK_COLS = NMOV + PSH        # 384 x cols + 1152 T cols
DMA_GROUPS = [1] * 12 + [4] * 6 + [5]
RING = 8
WARM_MMS = 26

_cache = {}


def _row_set(c):
    """y rows owned by core c; mirror-closed so y->95-y reverses the list."""
    return list(range(6 * c, 6 * c + 6)) + list(range(90 - 6 * c, 96 - 6 * c))


def _build_T_quarter():
    """T rows for angles i<108, detector d<48: (5184, 9216) float32."""
    # --- ramp filter as a circular-convolution matrix (filt = sino @ F) ---
    n = np.concatenate((np.arange(1, PAD // 2 + 1, 2), np.arange(PAD // 2 - 1, 0, -2)))
    f = np.zeros(PAD)
    f[0] = 0.25
    f[1::2] = -1.0 / (np.pi * n) ** 2
    full = 2.0 * np.real(np.fft.fft(f))
    ramp_bins = full[: PAD // 2 + 1].astype(np.float32).astype(np.float64)
    kern = np.fft.irfft(ramp_bins, n=PAD)
    s = np.pi / (2.0 * N_ANGLES)
    jj = np.arange(DET)[:, None]
    ii = np.arange(D_HALF)[None, :]
    F = (s * kern[(ii - jj) % PAD]).astype(np.float32)       # (DET j_in, 48 d_out)

    # --- backprojection weights as hat functions: W[a,d,p] = relu(1-|d-uc|)*inb ---
    angles = np.linspace(0.0, np.pi, N_ANGLES).astype(np.float32).astype(np.float64)[:A_HALF]
    grid = np.arange(WIDTH) - (WIDTH - 1) / 2.0
    ys, xs = np.meshgrid(grid, grid, indexing="ij")
    t = xs[None] * np.cos(angles)[:, None, None] + ys[None] * np.sin(angles)[:, None, None]
    u = t + (DET - 1) / 2.0                                  # (108, W, W)
    inb = ((u >= 0.0) & (u <= DET - 1)).astype(np.float32)
    uc = np.clip(u, 0.0, DET - 1).astype(np.float32)
    uc_flat = uc.reshape(A_HALF, P_TOTAL) * inb.reshape(A_HALF, P_TOTAL)
    inb_flat = inb.reshape(A_HALF, P_TOTAL)
    d = np.arange(DET, dtype=np.float32)
    T1 = np.empty((A_HALF, D_HALF, P_TOTAL), dtype=np.float32)
    for a in range(A_HALF):
        Wa = np.maximum(0.0, 1.0 - np.abs(d[:, None] - uc_flat[a][None, :])) * inb_flat[a][None, :]
        T1[a] = F.T @ Wa                                     # rows j = filtered-d 0..47

    # --- flip both spatial dims ---
    T1 = T1.reshape(A_HALF, D_HALF, WIDTH, WIDTH)[:, :, ::-1, ::-1]

    # --- upsample(1.8, linear, align_corners=False) + center-crop as one matrix ---
    up = int(WIDTH * UPSAMPLE)
    crop = (up - WIDTH) // 2
    coords = (np.arange(up) + 0.5) * (WIDTH / up) - 0.5
    coords = np.clip(coords, 0.0, WIDTH - 1)
    i0 = np.floor(coords).astype(np.int64)
    i1 = np.minimum(i0 + 1, WIDTH - 1)
    w = (coords - i0).astype(np.float32)
    C = np.zeros((WIDTH, up), dtype=np.float32)
    np.add.at(C, (i0, np.arange(up)), 1.0 - w)
    np.add.at(C, (i1, np.arange(up)), w)
    C = np.ascontiguousarray(C[:, crop : crop + WIDTH])      # (y in, Y out)

    T2 = np.tensordot(T1, C, axes=([2], [0]))                # (108, 48, X, Y)
    T2 = np.tensordot(T2, C, axes=([2], [0]))                # (108, 48, Y, X)
    return T2.reshape(KQ, P_TOTAL)


def _build_bass():
    import concourse.bass as bass
    import concourse.mybir as mybir
    from contextlib import ExitStack

    g_starts = np.cumsum([0] + DMA_GROUPS[:-1]).tolist()     # group start chunks
    NG = len(DMA_GROUPS)

    nc = bass.Bass()
    xtt = nc.declare_dram_parameter("xtt", [KCQ, 128, CHUNK_COLS], mybir.dt.bfloat16, isOutput=False)
    out = nc.declare_dram_parameter("out", [128, NBLK, NMOV], mybir.dt.bfloat16, isOutput=True)

    with ExitStack() as stack:
        u_sb = stack.enter_context(nc.sbuf_tensor([128, KCQ, CHUNK_COLS], mybir.dt.bfloat16))
        o_sb = stack.enter_context(nc.sbuf_tensor([128, NBLK, NMOV], mybir.dt.bfloat16))
        ps = [
            stack.enter_context(nc.psum_tensor(f"ps{i}", [128, 512], mybir.dt.float32))
            for i in range(8)
        ]
        scratch = stack.enter_context(nc.sbuf_tensor([128, 512], mybir.dt.bfloat16))
        c0_sems = [stack.enter_context(nc.semaphore(f"c0_sem{i}")) for i in range(3)]
        ring = [stack.enter_context(nc.semaphore(f"ring{b}")) for b in range(RING)]
        pe_sem = stack.enter_context(nc.semaphore("pe_sem"))
        copy_sem = stack.enter_context(nc.semaphore("copy_sem"))
        out_sem = stack.enter_context(nc.semaphore("out_sem"))
        block = stack.enter_context(nc.Block())

        # chunk-0 piece boundaries: [x + blk0 | blk1-3 | blk4-8]; each piece has
        # its OWN semaphore - two in-flight DMAs sharing a counter cannot be
        # ordered (per-SDMA-engine completions interleave)
        C0 = [0, NMOV + 128, NMOV + 4 * 128, CHUNK_COLS]

        @block.sync
        def _(s):
            for i in range(3):
                s.dma_start(
                    out=u_sb[:, 0:1, C0[i] : C0[i + 1]],
                    in_=xtt[0:1, :, C0[i] : C0[i + 1]].rearrange("k p n -> p k n"),
                ).then_inc(c0_sems[i], 16)
            for j, (gs, gl) in enumerate(zip(g_starts[1:], DMA_GROUPS[1:])):
                if j >= RING:
                    # ring-slot reuse: prior DMA on this semaphore must be done
                    s.wait_ge(ring[j % RING], (j // RING) * 16)
                s.dma_start(
                    out=u_sb[:, gs : gs + gl],
                    in_=xtt[gs : gs + gl].rearrange("k p n -> p k n"),
                ).then_inc(ring[j % RING], 16)
            # blocks 0-7 leave as one batched DMA, block 8 rides alone
            s.wait_ge(copy_sem, 8)
            s.dma_start(out=out[:, 0:8], in_=o_sb[:, 0:8]).then_inc(out_sem, 16)
            s.wait_ge(copy_sem, 9)
            s.dma_start(out=out[:, 8], in_=o_sb[:, 8]).then_inc(out_sem, 16)
            s.wait_ge(out_sem, 32)

        @block.tensor
        def _(te):
            # HAM warm-up while the first chunk is in flight: junk matmuls into
            # the spare [384:512] region of bank 7 that nothing ever reads.
            # scratch is uninitialized - junk values are fine, the result is
            # never read and the spare region's has_written bits are moot.
            for _ in range(WARM_MMS):
                nc.tensor.matmul(
                    ps[7][:, 384:512], scratch[:, 0:128], scratch[:, 0:128],
                    start=True, stop=True, skip_group_check=True,
                )
            last = None
            for c in range(KCQ):
                if c == 0:
                    te.wait_ge(c0_sems[0], 16)
                elif c in g_starts:
                    j = g_starts.index(c) - 1
                    te.wait_ge(ring[j % RING], (j // RING + 1) * 16)
                for blk in range(8):
                    if c == 0 and blk == 1:
                        te.wait_ge(c0_sems[1], 16)
                    if c == 0 and blk == 4:
                        te.wait_ge(c0_sems[2], 16)
                    last = nc.tensor.matmul(
                        ps[blk][:, 0:NMOV],
                        u_sb[:, c, NMOV + blk * 128 : NMOV + (blk + 1) * 128],
                        u_sb[:, c, 0:NMOV],
                        start=(c == 0),
                        stop=(c == KCQ - 1),
                        skip_group_check=True,
                    )
            last.then_inc(pe_sem, 1)
            # block 8 reuses bank 0 once its main-pass copy-out is done
            te.wait_ge(copy_sem, 1)
            for c in range(KCQ):
                last = nc.tensor.matmul(
                    ps[0][:, 0:NMOV],
                    u_sb[:, c, NMOV + 8 * 128 : NMOV + 9 * 128],
                    u_sb[:, c, 0:NMOV],
                    start=(c == 0),
                    stop=(c == KCQ - 1),
                    skip_group_check=True,
                )
            last.then_inc(pe_sem, 1)

        @block.vector
        def _(v):
            v.wait_ge(pe_sem, 1)
            for blk in range(8):
                # bank 0 first: the PE's block-8 pass is gated on copy_sem>=1
                nc.vector.tensor_copy(
                    o_sb[:, blk], ps[blk][:, 0:NMOV]
                ).then_inc(copy_sem, 1)
            v.wait_ge(pe_sem, 2)
            nc.vector.tensor_copy(
                o_sb[:, 8], ps[0][:, 0:NMOV]
            ).then_inc(copy_sem, 1)

    return nc


def _get_state():
    if "state" not in _cache:
        T = _build_T_quarter()
        t_bf = np.zeros((KQP, P_TOTAL), dtype=ml_dtypes.bfloat16)
        t_bf[:KQ] = T.astype(ml_dtypes.bfloat16)
        t_bf = t_bf.reshape(KCQ, 128, P_TOTAL)
        bufs = []
        for c in range(NCORES):
            cols = np.array(
                [y * WIDTH + x for y in _row_set(c) for x in range(WIDTH)], dtype=np.int64
            )
            buf = np.empty((KCQ, 128, CHUNK_COLS), dtype=ml_dtypes.bfloat16)
            buf[:, :, NMOV:] = t_bf[:, :, cols]
            bufs.append(buf)
        _cache["state"] = (bufs, _build_bass())
    return _cache["state"]


def _pack_lhsT(x_cols):
    """(SLICES, KQ) -> (128, KCQ, SLICES) zero-padded to KQP rows."""
    xp = np.zeros((SLICES, KQP), dtype=x_cols.dtype)
    xp[:, :KQ] = x_cols
    return xp.T.reshape(KCQ, 128, SLICES).transpose(1, 0, 2)


def _make_xt(x_flat):
    """(KCQ, 128, 4 variants x 96 slices) bf16 chunk packs."""
    v = x_flat.reshape(SLICES, N_ANGLES, DET)
    vr = v[:, ::-1]                                     # angle 215-i at block i
    xA = v[:, :A_HALF, :D_HALF].reshape(SLICES, KQ)
    xB = vr[:, :A_HALF, :D_HALF].reshape(SLICES, KQ)
    xC = v[:, :A_HALF, ::-1][:, :, :D_HALF].reshape(SLICES, KQ)   # d -> 95-d
    xD = vr[:, :A_HALF, ::-1][:, :, :D_HALF].reshape(SLICES, KQ)
    packs = [_pack_lhsT(q) for q in (xA, xB, xC, xD)]   # each (128, KCQ, 96)
    return np.ascontiguousarray(
        np.stack(packs, axis=2).reshape(128, KCQ, NMOV).transpose(1, 0, 2)
    ).astype(ml_dtypes.bfloat16)


def kernel(x, encoder_input_dims=None, decoder_target_shape=None, _want_perf=False):
    from concourse.bass_utils import run_bass_kernel_spmd

    bufs, nc = _get_state()
    x = np.asarray(x, dtype=np.float32)
    xt_host = _make_xt(x.reshape(SLICES, K))
    for c in range(NCORES):
        bufs[c][:, :, :NMOV] = xt_host
    in_maps = [{"xtt": bufs[c]} for c in range(NCORES)]
    res = run_bass_kernel_spmd(
        nc, in_maps, core_ids=list(range(NCORES)), trace=_want_perf
    )
    out = np.empty((SLICES, WIDTH, WIDTH), dtype=np.float32)
    for c in range(NCORES):
        r = np.asarray(res.results[c]["out"]).astype(np.float32)   # (128, 9, 384)
        v = r.transpose(1, 0, 2).reshape(PSH, NV, SLICES)
        g = lambda M: M.reshape(12, WIDTH, SLICES)       # (row_t, x, slice)
        o = (
            g(v[:, 0])
            + g(v[:, 1])[:, ::-1]                        # B: mirror_x
            + g(v[:, 2])[::-1, ::-1]                     # C: rot180
            + g(v[:, 3])[::-1]                           # D: mirror_y
        )
        for t, y in enumerate(_row_set(c)):
            out[:, y, :] = o[t].T
    out = out.reshape(2, 1, 48, WIDTH, WIDTH)
    if _want_perf:
        return out, res
    return out


# revision 30
# speedup vs baseline: 1.1669x; 1.1669x over previous
"""Fused FBP (ramp-filter + backprojection + flip + resize + crop) Trainium2 kernel.

The whole reference pipeline is linear in the input sinogram, so it folds into a
single constant matrix T of shape (A*DET, W*W) = (20736, 9216):

    out[n, p] = sum_k x_flat[n, k] * T[k, p]

T has a 4-fold exact symmetry:
  angle mirror:    T[(215-i, d)]    = mirror_x(T[(i, d)])        (i < 108)
  detector mirror: T[(i, 95-d)]     = rot180(T[(i, d)])          (d < 48)
so only the (i < 108, d < 48) quarter of T is streamed. The output-pixel axis is
sharded across 8 cores as y-mirror-closed row sets L_c = {6c..6c+5} u {90-6c..
95-6c}; each core owns 1152 pixels = 9 blocks of 128.

Mapping: T is the STATIONARY matmul operand (pixel block = PSUM partitions,
M=128, full array) and the four symmetry-variant x packs stream together as one
N=384 moving operand per (k-chunk, block):

    psum[blk][p, 4*96] += T[k, p_blk].T @ [xA | xB | xC | xD][k, :]

41 k-chunks accumulate per block; blocks 0-7 live in PSUM banks 0-7 chunk-outer
(so DMA stays ahead of the PE from the first chunk), block 8 runs as a second
41-chunk pass reusing bank 0 after its copy-out. The host applies the mirror
permutations when combining the four variant outputs:

    out[p] = A[p] + B[mirror_x(p)] + C[rot180(p)] + D[mirror_y(p)]

which is pure numpy indexing and off the measured HW path. PE cost: 369 matmuls
x (384/2.4 + 2.5) ns ~= 60 us vs ~85 us for the x-stationary formulation (the
moving operand there is T itself: 4 uses x 41x1152 columns = 188928 cycles).
"""

import numpy as np
import ml_dtypes

N_ANGLES = 216
DET = 96
WIDTH = 96
UPSAMPLE = 1.8
PAD = 256

SLICES = 96                    # 2*1*48 sinogram slices
K = N_ANGLES * DET             # 20736 full contraction length
P_TOTAL = WIDTH * WIDTH        # 9216 output pixels per slice
NCORES = 8
PSH = P_TOTAL // NCORES        # 1152 output pixels per core
NBLK = PSH // 128              # 9 pixel blocks of 128 per core
A_HALF = N_ANGLES // 2         # 108
D_HALF = DET // 2              # 48
KQ = A_HALF * D_HALF           # 5184 quarter rows
KCQ = (KQ + 127) // 128        # 41 k-chunks (last one zero-padded)
KQP = KCQ * 128                # 5248 padded rows
NV = 4                         # symmetry variants A,B,C,D
NMOV = NV * SLICES             # 384 moving columns per matmul

# The x pack and T slab for each chunk are fused into one DRAM row of
# NMOV+PSH=1536 bf16 cols, so one dma_start supplies a whole chunk and
# arrivals exactly track consumption. Per-chunk DMAs early (fine-grained
# pipeline), 4-chunk groups later (queue overhead amortized).
CHUNK_COLS = NMOV + PSH        # 384 x cols + 1152 T cols
DMA_GROUPS = [1] * 12 + [4] * 6 + [5]
RING = 8
WARM_MMS = 26

_cache = {}


def _row_set(c):
    """y rows owned by core c; mirror-closed so y->95-y reverses the list."""
    return list(range(6 * c, 6 * c + 6)) + list(range(90 - 6 * c, 96 - 6 * c))


def _build_T_quarter():
    """T rows for angles i<108, detector d<48: (5184, 9216) float32."""
    # --- ramp filter as a circular-convolution matrix (filt = sino @ F) ---
    n = np.concatenate((np.arange(1, PAD // 2 + 1, 2), np.arange(PAD // 2 - 1, 0, -2)))
    f = np.zeros(PAD)
    f[0] = 0.25
    f[1::2] = -1.0 / (np.pi * n) ** 2
    full = 2.0 * np.real(np.fft.fft(f))
    ramp_bins = full[: PAD // 2 + 1].astype(np.float32).astype(np.float64)
    kern = np.fft.irfft(ramp_bins, n=PAD)
    s = np.pi / (2.0 * N_ANGLES)
    jj = np.arange(DET)[:, None]
    ii = np.arange(D_HALF)[None, :]
    F = (s * kern[(ii - jj) % PAD]).astype(np.float32)       # (DET j_in, 48 d_out)

    # --- backprojection weights as hat functions: W[a,d,p] = relu(1-|d-uc|)*inb ---
    angles = np.linspace(0.0, np.pi, N_ANGLES).astype(np.float32).astype(np.float64)[:A_HALF]
    grid = np.arange(WIDTH) - (WIDTH - 1) / 2.0
    ys, xs = np.meshgrid(grid, grid, indexing="ij")
    t = xs[None] * np.cos(angles)[:, None, None] + ys[None] * np.sin(angles)[:, None, None]
    u = t + (DET - 1) / 2.0                                  # (108, W, W)
    inb = ((u >= 0.0) & (u <= DET - 1)).astype(np.float32)
    uc = np.clip(u, 0.0, DET - 1).astype(np.float32)
    uc_flat = uc.reshape(A_HALF, P_TOTAL) * inb.reshape(A_HALF, P_TOTAL)
    inb_flat = inb.reshape(A_HALF, P_TOTAL)
    d = np.arange(DET, dtype=np.float32)
    T1 = np.empty((A_HALF, D_HALF, P_TOTAL), dtype=np.float32)
    for a in range(A_HALF):
        Wa = np.maximum(0.0, 1.0 - np.abs(d[:, None] - uc_flat[a][None, :])) * inb_flat[a][None, :]
        T1[a] = F.T @ Wa                                     # rows j = filtered-d 0..47

    # --- flip both spatial dims ---
    T1 = T1.reshape(A_HALF, D_HALF, WIDTH, WIDTH)[:, :, ::-1, ::-1]

    # --- upsample(1.8, linear, align_corners=False) + center-crop as one matrix ---
    up = int(WIDTH * UPSAMPLE)
    crop = (up - WIDTH) // 2
    coords = (np.arange(up) + 0.5) * (WIDTH / up) - 0.5
    coords = np.clip(coords, 0.0, WIDTH - 1)
    i0 = np.floor(coords).astype(np.int64)
    i1 = np.minimum(i0 + 1, WIDTH - 1)
    w = (coords - i0).astype(np.float32)
    C = np.zeros((WIDTH, up), dtype=np.float32)
    np.add.at(C, (i0, np.arange(up)), 1.0 - w)
    np.add.at(C, (i1, np.arange(up)), w)
    C = np.ascontiguousarray(C[:, crop : crop + WIDTH])      # (y in, Y out)

    T2 = np.tensordot(T1, C, axes=([2], [0]))                # (108, 48, X, Y)
    T2 = np.tensordot(T2, C, axes=([2], [0]))                # (108, 48, Y, X)
    return T2.reshape(KQ, P_TOTAL)


def _build_bass():
    import concourse.bass as bass
    import concourse.mybir as mybir
    from contextlib import ExitStack

    g_starts = np.cumsum([0] + DMA_GROUPS[:-1]).tolist()     # group start chunks
    NG = len(DMA_GROUPS)

    nc = bass.Bass()
    xtt = nc.declare_dram_parameter("xtt", [KCQ, 128, CHUNK_COLS], mybir.dt.bfloat16, isOutput=False)
    out = nc.declare_dram_parameter("out", [128, NBLK, NMOV], mybir.dt.bfloat16, isOutput=True)

    with ExitStack() as stack:
        u_sb = stack.enter_context(nc.sbuf_tensor([128, KCQ, CHUNK_COLS], mybir.dt.bfloat16))
        o_sb = stack.enter_context(nc.sbuf_tensor([128, NBLK, NMOV], mybir.dt.bfloat16))
        ps = [
            stack.enter_context(nc.psum_tensor(f"ps{i}", [128, 512], mybir.dt.float32))
            for i in range(8)
        ]
        scratch = stack.enter_context(nc.sbuf_tensor([128, 512], mybir.dt.bfloat16))
        c0_sems = [stack.enter_context(nc.semaphore(f"c0_sem{i}")) for i in range(3)]
        ring = [stack.enter_context(nc.semaphore(f"ring{b}")) for b in range(RING)]
        pe_sem = stack.enter_context(nc.semaphore("pe_sem"))
        copy_sem = stack.enter_context(nc.semaphore("copy_sem"))
        out_sem = stack.enter_context(nc.semaphore("out_sem"))
        block = stack.enter_context(nc.Block())

        # chunk-0 piece boundaries: [x + blk0 | blk1-3 | blk4-8]; each piece has
        # its OWN semaphore - two in-flight DMAs sharing a counter cannot be
        # ordered (per-SDMA-engine completions interleave)
        C0 = [0, NMOV + 128, NMOV + 4 * 128, CHUNK_COLS]

        @block.sync
        def _(s):
            for i in range(3):
                s.dma_start(
                    out=u_sb[:, 0:1, C0[i] : C0[i + 1]],
                    in_=xtt[0:1, :, C0[i] : C0[i + 1]].rearrange("k p n -> p k n"),
                ).then_inc(c0_sems[i], 16)
            for j, (gs, gl) in enumerate(zip(g_starts[1:], DMA_GROUPS[1:])):
                if j >= RING:
                    # ring-slot reuse: prior DMA on this semaphore must be done
                    s.wait_ge(ring[j % RING], (j // RING) * 16)
                s.dma_start(
                    out=u_sb[:, gs : gs + gl],
                    in_=xtt[gs : gs + gl].rearrange("k p n -> p k n"),
                ).then_inc(ring[j % RING], 16)
            # blocks 0-7 leave as one batched DMA, block 8 in two halves
            s.wait_ge(copy_sem, 8)
            s.dma_start(out=out[:, 0:8], in_=o_sb[:, 0:8]).then_inc(out_sem, 16)
            s.wait_ge(copy_sem, 9)
            s.dma_start(out=out[:, 8, 0 : NMOV // 2], in_=o_sb[:, 8, 0 : NMOV // 2]).then_inc(out_sem, 16)
            s.wait_ge(copy_sem, 10)
            s.dma_start(out=out[:, 8, NMOV // 2 : NMOV], in_=o_sb[:, 8, NMOV // 2 : NMOV]).then_inc(out_sem, 16)
            s.wait_ge(out_sem, 48)

        @block.tensor
        def _(te):
            # HAM warm-up while the first chunk is in flight: junk matmuls into
            # the spare [384:512] region of bank 7 that nothing ever reads.
            # scratch is uninitialized - junk values are fine, the result is
            # never read and the spare region's has_written bits are moot.
            for _ in range(WARM_MMS):
                nc.tensor.matmul(
                    ps[7][:, 384:512], scratch[:, 0:128], scratch[:, 0:128],
                    start=True, stop=True, skip_group_check=True,
                )
            last = None
            for c in range(KCQ):
                if c == 0:
                    te.wait_ge(c0_sems[0], 16)
                elif c in g_starts:
                    j = g_starts.index(c) - 1
                    te.wait_ge(ring[j % RING], (j // RING + 1) * 16)
                for blk in range(8):
                    if c == 0 and blk == 1:
                        te.wait_ge(c0_sems[1], 16)
                    if c == 0 and blk == 4:
                        te.wait_ge(c0_sems[2], 16)
                    mm = nc.tensor.matmul(
                        ps[blk][:, 0:NMOV],
                        u_sb[:, c, NMOV + blk * 128 : NMOV + (blk + 1) * 128],
                        u_sb[:, c, 0:NMOV],
                        start=(c == 0),
                        stop=(c == KCQ - 1),
                        skip_group_check=True,
                    )
                    if c == KCQ - 1:
                        # per-block inc on the last chunk: bank b's copy-out
                        # starts while the PE finishes blocks b+1..7
                        mm.then_inc(pe_sem, 1)
            # block 8 reuses bank 0 once its main-pass copy-out is done
            te.wait_ge(copy_sem, 1)
            for c in range(KCQ):
                last = nc.tensor.matmul(
                    ps[0][:, 0:NMOV],
                    u_sb[:, c, NMOV + 8 * 128 : NMOV + 9 * 128],
                    u_sb[:, c, 0:NMOV],
                    start=(c == 0),
                    stop=(c == KCQ - 1),
                    skip_group_check=True,
                )
            last.then_inc(pe_sem, 1)

        @block.vector
        def _(v):
            for blk in range(8):
                # bank b's copy chases the PE through the last chunk;
                # bank 0 first: the PE's block-8 pass is gated on copy_sem>=1
                v.wait_ge(pe_sem, blk + 1)
                nc.vector.tensor_copy(
                    o_sb[:, blk], ps[blk][:, 0:NMOV]
                ).then_inc(copy_sem, 1)
            v.wait_ge(pe_sem, 9)
            # block 8 leaves in two pipelined halves (copy1 | dma1+copy2 | dma2)
            nc.vector.tensor_copy(
                o_sb[:, 8, 0 : NMOV // 2], ps[0][:, 0 : NMOV // 2]
            ).then_inc(copy_sem, 1)
            nc.vector.tensor_copy(
                o_sb[:, 8, NMOV // 2 : NMOV], ps[0][:, NMOV // 2 : NMOV]
            ).then_inc(copy_sem, 1)

    return nc


def _get_state():
    if "state" not in _cache:
        T = _build_T_quarter()
        t_bf = np.zeros((KQP, P_TOTAL), dtype=ml_dtypes.bfloat16)
        t_bf[:KQ] = T.astype(ml_dtypes.bfloat16)
        t_bf = t_bf.reshape(KCQ, 128, P_TOTAL)
        bufs = []
        for c in range(NCORES):
            cols = np.array(
                [y * WIDTH + x for y in _row_set(c) for x in range(WIDTH)], dtype=np.int64
            )
            buf = np.empty((KCQ, 128, CHUNK_COLS), dtype=ml_dtypes.bfloat16)
            buf[:, :, NMOV:] = t_bf[:, :, cols]
            bufs.append(buf)
        _cache["state"] = (bufs, _build_bass())
    return _cache["state"]


def _pack_lhsT(x_cols):
    """(SLICES, KQ) -> (128, KCQ, SLICES) zero-padded to KQP rows."""
    xp = np.zeros((SLICES, KQP), dtype=x_cols.dtype)
    xp[:, :KQ] = x_cols
    return xp.T.reshape(KCQ, 128, SLICES).transpose(1, 0, 2)


def _make_xt(x_flat):
    """(KCQ, 128, 4 variants x 96 slices) bf16 chunk packs."""
    v = x_flat.reshape(SLICES, N_ANGLES, DET)
    vr = v[:, ::-1]                                     # angle 215-i at block i
    xA = v[:, :A_HALF, :D_HALF].reshape(SLICES, KQ)
    xB = vr[:, :A_HALF, :D_HALF].reshape(SLICES, KQ)
    xC = v[:, :A_HALF, ::-1][:, :, :D_HALF].reshape(SLICES, KQ)   # d -> 95-d
    xD = vr[:, :A_HALF, ::-1][:, :, :D_HALF].reshape(SLICES, KQ)
    packs = [_pack_lhsT(q) for q in (xA, xB, xC, xD)]   # each (128, KCQ, 96)
    return np.ascontiguousarray(
        np.stack(packs, axis=2).reshape(128, KCQ, NMOV).transpose(1, 0, 2)
    ).astype(ml_dtypes.bfloat16)


def kernel(x, encoder_input_dims=None, decoder_target_shape=None, _want_perf=False):
    from concourse.bass_utils import run_bass_kernel_spmd

    bufs, nc = _get_state()
    x = np.asarray(x, dtype=np.float32)
    xt_host = _make_xt(x.reshape(SLICES, K))
    for c in range(NCORES):
        bufs[c][:, :, :NMOV] = xt_host
    in_maps = [{"xtt": bufs[c]} for c in range(NCORES)]
    res = run_bass_kernel_spmd(
        nc, in_maps, core_ids=list(range(NCORES)), trace=_want_perf
    )
    out = np.empty((SLICES, WIDTH, WIDTH), dtype=np.float32)
    for c in range(NCORES):
        r = np.asarray(res.results[c]["out"]).astype(np.float32)   # (128, 9, 384)
        v = r.transpose(1, 0, 2).reshape(PSH, NV, SLICES)
        g = lambda M: M.reshape(12, WIDTH, SLICES)       # (row_t, x, slice)
        o = (
            g(v[:, 0])
            + g(v[:, 1])[:, ::-1]                        # B: mirror_x
            + g(v[:, 2])[::-1, ::-1]                     # C: rot180
            + g(v[:, 3])[::-1]                           # D: mirror_y
        )
        for t, y in enumerate(_row_set(c)):
            out[:, y, :] = o[t].T
    out = out.reshape(2, 1, 48, WIDTH, WIDTH)
    if _want_perf:
        return out, res
    return out
